# revision 10
# baseline (speedup 1.0000x reference)
"""Trainium2 Bass kernel for nn_AirResistance.

out[b, t] = x[b, 0] * r**t,  r = 1 + (0.99 - 1.0) * delta_t,  out: (B, steps, 1) f32

Rank-1 structure: out = x ⊗ rpow. The power vector rpow is precomputed on the
host as one 16KB row; on-chip it is broadcast to all 128 partitions with a
K=1 PE outer product (ones ⊗ rpow) into PSUM, then copied to SBUF by the
otherwise-idle scalar engine. Output values are produced with per-partition-
scalar multiplies on the vector engine and streamed to HBM. Batch dim B is
sharded across the 8 NeuronCores (pure data parallelism, no communication).

Raw Bass (manual semaphores): this toolchain's walrus enforces at most one
sync-wait command per instruction, so waits are standalone wait_ge
instructions and every producer increments exactly one semaphore. Slot reuse
is gated by per-slot semaphores (a single shared completion counter would
race: DMA completions interleave per-engine across transfers).

DMA layout: HWDGE fans one descriptor per SBUF partition across the 16 SDMA
engines, and engine 15 has a fixed per-descriptor handicap that makes it the
kernel straggler with 16KB descriptors. Steady-state groups cover 512 output
rows with partition p holding rows 4p..4p+3 (contiguous 64KB in DRAM and
SBUF), giving 64KB descriptors that amortize the handicap while keeping all
16 engines loaded at line rate.

Ramp: group 0 computes straight out of PSUM in column chunks as each matmul
bank lands, and the first groups are small (128/128/256 rows) with dedicated
SBUF slots, so output DMAs start as soon as the first banks of the broadcast
finish instead of after a full 2MiB table load.
"""

import numpy as np

import concourse.bass as bass
from concourse import mybir
from concourse.bass_utils import run_bass_kernel_spmd

N_CORES = 8
B = 32768
STEPS = 4096
P = 128
BANK = 512                            # fp32 PSUM bank = 512 cols
N_BANKS = STEPS // BANK               # 8
ROWS_PER_CORE = B // N_CORES          # 4096
K = 2                                 # steady SBUF slots (64KB/partition each)
MAX_RPP = 4
G0_CHUNK = 1024                       # group-0 column-chunk width
N_G0_CHUNKS = STEPS // G0_CHUNK       # 4

# groups: rpp (rows = 128*rpp); g0 is column-chunked, g0/g1 use dedicated slots
_GROUP_RPP = [1, 1, 2] + [4] * 7
assert sum(_GROUP_RPP) * P == ROWS_PER_CORE

_nc_cache = None


def _group_meta():
    metas = []
    row0 = 0
    col0 = 0
    for g, rpp in enumerate(_GROUP_RPP):
        metas.append({"row0": row0, "rpp": rpp, "xt_col0": col0})
        row0 += P * rpp
        col0 += rpp
    return metas


def _build_bass():
    f32 = mybir.dt.float32
    nc = bass.Bass("TRN2", target_bir_lowering=False, debug=False)

    metas = _group_meta()
    n_xt_cols = sum(m["rpp"] for m in metas)
    n_groups = len(metas)

    xt_d = nc.dram_tensor("xt", [P, n_xt_cols], f32, kind="ExternalInput").ap()
    rp1_d = nc.dram_tensor("rp1", [1, STEPS], f32, kind="ExternalInput").ap()
    out_d = nc.dram_tensor(
        "out", [ROWS_PER_CORE, STEPS], f32, kind="ExternalOutput"
    ).ap()

    xt_sb = nc.alloc_sbuf_tensor("xt_sb", [P, n_xt_cols], f32).ap()
    rp1_sb = nc.alloc_sbuf_tensor("rp1_sb", [1, STEPS], f32).ap()
    ones_sb = nc.alloc_sbuf_tensor("ones_sb", [1, P], f32).ap()
    rp_sb = nc.alloc_sbuf_tensor("rp_sb", [P, STEPS], f32).ap()
    rp_ps = nc.alloc_psum_tensor("rp_ps", [P, STEPS], f32).ap()
    # steady slots + two dedicated ramp slots (16KB each)
    ot_sb = nc.alloc_sbuf_tensor("ot_sb", [P, K, MAX_RPP, STEPS], f32).ap()
    e0_sb = nc.alloc_sbuf_tensor("e0_sb", [P, STEPS], f32).ap()
    e1_sb = nc.alloc_sbuf_tensor("e1_sb", [P, STEPS], f32).ap()

    def group_ot(g, m):
        if g == 0:
            return e0_sb.rearrange("p (j t) -> p j t", j=1)
        if g == 1:
            return e1_sb.rearrange("p (j t) -> p j t", j=1)
        return ot_sb[:, g % K, :, :]

    def out_ap(m, j0, j1, c0, c1):
        rpp = m["rpp"]
        g_rows = out_d[m["row0"] : m["row0"] + P * rpp, :]
        g3 = g_rows.rearrange("(p j) t -> p j t", j=rpp)
        return g3[:, j0:j1, c0:c1]

    # sub-DMA lists per group: (j0, j1, c0, c1)
    subs_per_group = []
    for g, m in enumerate(metas):
        if g == 0:
            subs_per_group.append(
                [(0, 1, c * G0_CHUNK, (c + 1) * G0_CHUNK) for c in range(N_G0_CHUNKS)]
            )
        else:
            subs_per_group.append([(0, m["rpp"], 0, STEPS)])

    # per-slot cumulative DMA-inc totals (ramp slots share the steady sems
    # for final accounting; their DMAs still inc slot_sems[g % K])
    slot_after_group = {}
    run = {0: 0, 1: 0}
    for g, m in enumerate(metas):
        run[g % K] += 16 * len(subs_per_group[g])
        slot_after_group[g] = run[g % K]

    with (
        nc.Block() as block,
        nc.semaphore("sem_xt") as sem_xt,
        nc.semaphore("sem_r1") as sem_r1,
        nc.semaphore("sem_on") as sem_on,
        nc.semaphore("sem_mm") as sem_mm,
        nc.semaphore("sem_rp") as sem_rp,
        nc.semaphore("sem_cmp") as sem_cmp,
        nc.semaphore("sem_s0") as sem_s0,
        nc.semaphore("sem_s1") as sem_s1,
    ):
        slot_sems = [sem_s0, sem_s1]

        @block.gpsimd
        def _(gpsimd):
            gpsimd.memset(ones_sb, 1.0).then_inc(sem_on, 1)

        @block.tensor
        def _(tensor):
            tensor.wait_ge(sem_on, 1)
            tensor.wait_ge(sem_r1, 16)
            for c in range(N_BANKS):
                tensor.matmul(
                    rp_ps[:, c * BANK : (c + 1) * BANK],
                    ones_sb,
                    rp1_sb[:, c * BANK : (c + 1) * BANK],
                ).then_inc(sem_mm, 1)

        @block.scalar
        def _(scalar):
            for c in range(N_BANKS):
                scalar.wait_ge(sem_mm, c + 1)
                scalar.copy(
                    rp_sb[:, c * BANK : (c + 1) * BANK],
                    rp_ps[:, c * BANK : (c + 1) * BANK],
                ).then_inc(sem_rp, 1)

        @block.sync
        def _(sync):
            sync.dma_start(out=rp1_sb, in_=rp1_d).then_inc(sem_r1, 16)
            sync.dma_start(out=xt_sb, in_=xt_d).then_inc(sem_xt, 16)
            done_ts = 0
            for g, m in enumerate(metas):
                for j0, j1, c0, c1 in subs_per_group[g]:
                    done_ts += j1 - j0
                    sync.wait_ge(sem_cmp, done_ts)
                    sync.dma_start(
                        out=out_ap(m, j0, j1, c0, c1),
                        in_=group_ot(g, m)[:, j0:j1, c0:c1],
                    ).then_inc(slot_sems[g % K], 16)
            sync.wait_ge(sem_s0, slot_after_group[n_groups - 2])
            sync.wait_ge(sem_s1, slot_after_group[n_groups - 1])

        @block.vector
        def _(vector):
            vector.wait_ge(sem_xt, 16)
            for g, m in enumerate(metas):
                if g >= 4:
                    # slot (g % K) was last drained by the group-(g-K) DMAs
                    # (g0/g1 live in dedicated ramp slots, so g2/g3 skip this)
                    vector.wait_ge(slot_sems[g % K], slot_after_group[g - K])
                if g == 0:
                    # chunk by chunk as the scalar engine lands each PSUM bank
                    # in rp_sb (DVE must never read PSUM while PE still writes
                    # other banks — that combination crashes the hardware)
                    for c in range(N_G0_CHUNKS):
                        banks_needed = (c + 1) * G0_CHUNK // BANK
                        vector.wait_ge(sem_rp, banks_needed)
                        vector.tensor_scalar_mul(
                            group_ot(g, m)[:, 0, c * G0_CHUNK : (c + 1) * G0_CHUNK],
                            rp_sb[:, c * G0_CHUNK : (c + 1) * G0_CHUNK],
                            xt_sb[:, m["xt_col0"] : m["xt_col0"] + 1],
                        ).then_inc(sem_cmp, 1)
                    continue
                if g == 1:
                    vector.wait_ge(sem_rp, N_BANKS)  # rp_sb fully materialized
                for j in range(m["rpp"]):
                    vector.tensor_scalar_mul(
                        group_ot(g, m)[:, j, :],
                        rp_sb,
                        xt_sb[:, m["xt_col0"] + j : m["xt_col0"] + j + 1],
                    ).then_inc(sem_cmp, 1)

    return nc


def _get_nc():
    global _nc_cache
    if _nc_cache is None:
        _nc_cache = _build_bass()
    return _nc_cache


def make_in_maps(x, delta_t):
    x = np.asarray(x, dtype=np.float32)
    r32 = np.float32(1.0 + (0.99 - 1.0) * float(delta_t))
    rpow = (np.float64(r32) ** np.arange(STEPS, dtype=np.float64)).astype(np.float32)
    rp1 = np.ascontiguousarray(rpow.reshape(1, STEPS))

    metas = _group_meta()
    n_xt_cols = sum(m["rpp"] for m in metas)

    in_maps = []
    for c in range(N_CORES):
        xs = x[c * ROWS_PER_CORE : (c + 1) * ROWS_PER_CORE, 0]
        # xt[p, col0+j] = x_shard[row0 + rpp*p + j]
        xt = np.zeros((P, n_xt_cols), dtype=np.float32)
        for m in metas:
            rpp = m["rpp"]
            blk = xs[m["row0"] : m["row0"] + P * rpp].reshape(P, rpp)
            xt[:, m["xt_col0"] : m["xt_col0"] + rpp] = blk
        in_maps.append({"xt": xt, "rp1": rp1})
    return in_maps


def kernel(steps, x, delta_t):
    steps = int(steps)
    x = np.asarray(x, dtype=np.float32)
    assert steps == STEPS and x.shape == (B, 1), (steps, x.shape)

    res = run_bass_kernel_spmd(
        _get_nc(), make_in_maps(x, delta_t), list(range(N_CORES))
    )
    out = np.concatenate([res.results[c]["out"] for c in range(N_CORES)], axis=0)
    return out.reshape(B, STEPS, 1)


# revision 14
# speedup vs baseline: 1.0565x; 1.0565x over previous
"""Trainium2 Bass kernel for nn_AirResistance.

out[b, t] = x[b, 0] * r**t,  r = 1 + (0.99 - 1.0) * delta_t,  out: (B, steps, 1) f32

Rank-1 structure: out = x ⊗ rpow. The power vector rpow is precomputed on the
host (tiny) and broadcast to all 128 SBUF partitions; output values are
produced with per-partition-scalar multiplies on the vector engine and
streamed to HBM. Batch dim B is sharded across the 8 NeuronCores (pure data
parallelism, no communication).

Raw Bass (manual semaphores): this toolchain's walrus enforces at most one
sync-wait command per instruction, so waits are standalone wait_ge
instructions and every producer increments exactly one semaphore. Slot reuse
is gated by per-slot semaphores (a single shared completion counter would
race: DMA completions interleave per-engine across transfers).

DMA layout: HWDGE fans one descriptor per SBUF partition across the 16 SDMA
engines, and engine 15 has a fixed per-descriptor handicap that makes it the
kernel straggler with 16KB descriptors. Steady-state groups cover 512 output
rows with partition p holding rows 4p..4p+3 (contiguous 64KB in DRAM and
SBUF), giving 64KB descriptors that amortize the handicap while keeping all
16 engines loaded at line rate.

Ramp: the rp table loads as two column-half DMAs, and the first groups are
small (128/128/256 rows, with group 0 stored as two column-half DMAs), so the
first output DMA issues right after the first rp half lands instead of after
a full-table load plus a full 512-row group compute.
"""

import numpy as np

import concourse.bass as bass
from concourse import mybir
from concourse.bass_utils import run_bass_kernel_spmd

N_CORES = 8
B = 32768
STEPS = 4096
HALF = STEPS // 2
P = 128
ROWS_PER_CORE = B // N_CORES          # 4096
K = 2                                 # SBUF slots (64KB/partition each)
MAX_RPP = 4

# groups: (rpp, col_split) — rows = 128*rpp; col_split only for group 0
_GROUPS = [(1, True), (1, False), (2, False)] + [(4, False)] * 7
assert sum(r for r, _ in _GROUPS) * P == ROWS_PER_CORE

_nc_cache = None


def _group_meta():
    """Per group: row0, rpp, xt_col0, list of (j-range, col-range) sub-DMAs."""
    metas = []
    row0 = 0
    col0 = 0
    for rpp, col_split in _GROUPS:
        if col_split:
            subs = [(0, rpp, 0, HALF), (0, rpp, HALF, STEPS)]
        else:
            subs = [(0, rpp, 0, STEPS)]
        metas.append({"row0": row0, "rpp": rpp, "xt_col0": col0, "subs": subs})
        row0 += P * rpp
        col0 += rpp
    return metas


def _build_bass():
    f32 = mybir.dt.float32
    nc = bass.Bass("TRN2", target_bir_lowering=False, debug=False)

    metas = _group_meta()
    n_xt_cols = sum(m["rpp"] for m in metas)

    xt_d = nc.dram_tensor("xt", [P, n_xt_cols], f32, kind="ExternalInput").ap()
    rp_d = nc.dram_tensor("rp", [P, STEPS], f32, kind="ExternalInput").ap()
    out_d = nc.dram_tensor(
        "out", [ROWS_PER_CORE, STEPS], f32, kind="ExternalOutput"
    ).ap()

    rp_sb = nc.alloc_sbuf_tensor("rp_sb", [P, STEPS], f32).ap()
    xt_sb = nc.alloc_sbuf_tensor("xt_sb", [P, n_xt_cols], f32).ap()
    ot_sb = nc.alloc_sbuf_tensor("ot_sb", [P, K, MAX_RPP, STEPS], f32).ap()
    # dedicated ramp slots for groups 0/1 so groups 2/3 skip slot-reuse waits
    e0_sb = nc.alloc_sbuf_tensor("e0_sb", [P, 1, STEPS], f32).ap()
    e1_sb = nc.alloc_sbuf_tensor("e1_sb", [P, 1, STEPS], f32).ap()

    def group_ot(g):
        if g == 0:
            return e0_sb
        if g == 1:
            return e1_sb
        return ot_sb[:, g % K, :, :]

    # out AP for group g: partition p, row row0 + rpp*p + j, cols [c0:c1]
    def out_ap(m, j0, j1, c0, c1):
        rpp = m["rpp"]
        g_rows = out_d[m["row0"] : m["row0"] + P * rpp, :]
        # (p, j, t) with row = rpp*p + j
        g3 = g_rows.rearrange("(p j) t -> p j t", j=rpp)
        return g3[:, j0:j1, c0:c1]

    # TS op counts per group (for sem_cmp thresholds)
    ts_per_group = []
    for m in metas:
        n = 0
        for j0, j1, c0, c1 in m["subs"]:
            n += j1 - j0
        ts_per_group.append(n)
    cum_ts = np.concatenate([[0], np.cumsum(ts_per_group)])

    # per-slot cumulative DMA-inc totals
    slot_cum = {0: [], 1: []}  # list of cumulative inc counts after each group
    run = {0: 0, 1: 0}
    for g, m in enumerate(metas):
        run[g % K] += 16 * len(m["subs"])
        slot_cum[g % K].append(run[g % K])
    slot_after_group = {}  # group g -> slot sem value once its DMAs complete
    run = {0: 0, 1: 0}
    for g, m in enumerate(metas):
        run[g % K] += 16 * len(m["subs"])
        slot_after_group[g] = run[g % K]

    with (
        nc.Block() as block,
        nc.semaphore("sem_xt") as sem_xt,
        nc.semaphore("sem_rlo") as sem_rlo,
        nc.semaphore("sem_rhi") as sem_rhi,
        nc.semaphore("sem_cmp") as sem_cmp,
        nc.semaphore("sem_s0") as sem_s0,
        nc.semaphore("sem_s1") as sem_s1,
    ):
        slot_sems = [sem_s0, sem_s1]

        # group -> issuing queue: even groups on the SP HWDGE ring, odd on the
        # ACT HWDGE ring (two independent descriptor rings feed the SDMA
        # engines; splits per-ring FIFO pressure and hedges against per-core
        # slow engines behind one ring)
        def emit_group_dmas(eng, g, m, ts_before):
            done_ts = ts_before
            for j0, j1, c0, c1 in m["subs"]:
                done_ts += j1 - j0
                eng.wait_ge(sem_cmp, done_ts)
                eng.dma_start(
                    out=out_ap(m, j0, j1, c0, c1),
                    in_=group_ot(g)[:, j0:j1, c0:c1],
                ).then_inc(slot_sems[g % K], 16)

        @block.sync
        def _(sync):
            sync.dma_start(out=xt_sb, in_=xt_d).then_inc(sem_xt, 16)
            sync.dma_start(out=rp_sb[:, :HALF], in_=rp_d[:, :HALF]).then_inc(
                sem_rlo, 16
            )
            sync.dma_start(out=rp_sb[:, HALF:], in_=rp_d[:, HALF:]).then_inc(
                sem_rhi, 16
            )
            for g, m in enumerate(metas):
                if g % 2 == 0:
                    emit_group_dmas(sync, g, m, int(cum_ts[g]))
            sync.wait_ge(sem_s0, slot_after_group[len(metas) - 2])
            sync.wait_ge(sem_s1, slot_after_group[len(metas) - 1])

        @block.scalar
        def _(scalar):
            for g, m in enumerate(metas):
                if g % 2 == 1:
                    emit_group_dmas(scalar, g, m, int(cum_ts[g]))

        @block.vector
        def _(vector):
            vector.wait_ge(sem_xt, 16)
            vector.wait_ge(sem_rlo, 16)
            waited_rhi = False
            for g, m in enumerate(metas):
                if g >= 4:
                    # groups 0/1 live in dedicated ramp slots, so the first
                    # steady-slot users (groups 2/3) need no reuse wait
                    vector.wait_ge(slot_sems[g % K], slot_after_group[g - K])
                for j0, j1, c0, c1 in m["subs"]:
                    if c1 > HALF and not waited_rhi:
                        vector.wait_ge(sem_rhi, 16)
                        waited_rhi = True
                    for j in range(j0, j1):
                        vector.tensor_scalar_mul(
                            group_ot(g)[:, j, c0:c1],
                            rp_sb[:, c0:c1],
                            xt_sb[:, m["xt_col0"] + j : m["xt_col0"] + j + 1],
                        ).then_inc(sem_cmp, 1)

    return nc


def _get_nc():
    global _nc_cache
    if _nc_cache is None:
        _nc_cache = _build_bass()
    return _nc_cache


def make_in_maps(x, delta_t):
    x = np.asarray(x, dtype=np.float32)
    r32 = np.float32(1.0 + (0.99 - 1.0) * float(delta_t))
    rpow = (np.float64(r32) ** np.arange(STEPS, dtype=np.float64)).astype(np.float32)
    rp_b = np.ascontiguousarray(np.broadcast_to(rpow, (P, STEPS)))

    metas = _group_meta()
    n_xt_cols = sum(m["rpp"] for m in metas)

    in_maps = []
    for c in range(N_CORES):
        xs = x[c * ROWS_PER_CORE : (c + 1) * ROWS_PER_CORE, 0]
        # xt[p, col0+j] = x_shard[row0 + rpp*p + j]
        xt = np.zeros((P, n_xt_cols), dtype=np.float32)
        for m in metas:
            rpp = m["rpp"]
            blk = xs[m["row0"] : m["row0"] + P * rpp].reshape(P, rpp)
            xt[:, m["xt_col0"] : m["xt_col0"] + rpp] = blk
        in_maps.append({"xt": xt, "rp": rp_b})
    return in_maps


def kernel(steps, x, delta_t):
    steps = int(steps)
    x = np.asarray(x, dtype=np.float32)
    assert steps == STEPS and x.shape == (B, 1), (steps, x.shape)

    res = run_bass_kernel_spmd(
        _get_nc(), make_in_maps(x, delta_t), list(range(N_CORES))
    )
    out = np.concatenate([res.results[c]["out"] for c in range(N_CORES)], axis=0)
    return out.reshape(B, STEPS, 1)
